# revision 1
# baseline (speedup 1.0000x reference)
"""Batched min-variance weights kernel for Trainium2.

Problem: for 8192 SPD 64x64 covariance matrices S, compute
    w = S^-1 1 / (1^T S^-1 1),  output shape [8192, 64, 1], fp32.

Strategy: data-parallel across 8 NeuronCores (1024 matrices each).
Within a core: lane-parallel Gaussian elimination. Each SBUF partition
holds one full matrix in its free dimension, so a tile processes 128
matrices simultaneously. Elimination steps are broadcast-access-pattern
VectorE ops (scalar_tensor_tensor with a per-partition pivot-reciprocal
scalar), vectorized across all 128 lanes. No pivoting is required: the
matrices are SPD (pivots >= lambda_min >= 0.1).

Layout: the augmented system is stored transposed, [65 rows x 64 cols]
per matrix, with the RHS (ones vector) as row 64. Column elimination on
the symmetric system z^T S = 1^T is algebraically identical to row GE on
S z = 1, and keeps every DMA transfer and every elimination operand
contiguous in the free dimension.

Sign convention: substitution produces zn = -z; since the output is
z / sum(z), the sign cancels and no fixup is needed.

Hardware constraint honored throughout: every TPB instruction has exactly
one sync-wait slot, so the program is structured so Tile never needs two
semaphore waits on one instruction (single mega buffer, 2 loads + 1 store
total, per-tile VectorE touch copies as sole DMA-wait carriers, all
compute on VectorE in program order).
"""

import os

import numpy as np

B = 8192
N = 64
H = N + 1          # rows per stored matrix: 64 matrix rows + rhs row
NCORES = 8
BPC = B // NCORES  # 1024 matrices per core
P = 128            # matrices per tile (one per SBUF partition)
NT = BPC // P      # 8 tiles per core

_CACHE = {}

LAST_EXEC_NS = None


def _patch_tail_drain():
    """Split the kernel-tail drain's semaphore waits into standalone wait
    instructions. Every TPB instruction (the drain included) has exactly one
    hardware sync-wait slot; Tile's stock tail drain attaches one wait per
    outstanding DMA/engine semaphore and walrus rejects it."""
    import concourse.mybir as mybir
    import concourse.tile as tile_mod
    from concourse.bass import SemaphoreHandle
    from concourse.vector_clock import ScopedClock

    if getattr(tile_mod.TileContext, "_drain_split_patched", False):
        return

    def _drain_and_barrier(self, tick_clock, wait_clock):
        drain_inst = self.nc.sync.drain()
        wait_clock.add_sem_waits(
            drain_inst.ins, ScopedClock({None: tick_clock.global_clock})
        )
        si = drain_inst.ins.sync_info
        if si is not None and len(si.on_wait) > 1:
            waits = list(si.on_wait)
            drain_inst.ins.sync_info = mybir.SyncInfo(
                on_wait=[waits[0]], on_update=list(si.on_update)
            )
            for w in waits[1:]:
                self.nc.sync.wait_ge(
                    SemaphoreHandle(w.ant_name, w.id), w.wait_value
                )
        self.nc.all_engine_barrier()
        assert self.sems is not None
        popped = self.nc._tile_sem_poison_stack.pop()
        assert popped is self._sem_poison
        self.nc.clear_and_free_semaphores(list(self.sems.allocated().values()))
        self.nc.all_engine_barrier()

    tile_mod.TileContext._drain_and_barrier = _drain_and_barrier
    tile_mod.TileContext._drain_split_patched = True


def _build_program():
    import concourse.bass as bass
    import concourse.mybir as mybir
    from concourse.tile import TileContext

    _patch_tail_drain()

    fp32 = mybir.dt.float32
    Alu = mybir.AluOpType

    nc = bass.Bass()
    sigma = nc.dram_tensor("sigma", [BPC, N, N], fp32, kind="ExternalInput")
    wout = nc.dram_tensor("w", [BPC, N], fp32, kind="ExternalOutput")

    with TileContext(nc) as tc:
        with (
            tc.tile_pool(name="mpool", bufs=1) as mpool,
            tc.tile_pool(name="ptpool", bufs=2) as ptpool,
            tc.tile_pool(name="spool", bufs=2) as spool,
            tc.tile_pool(name="cpool", bufs=1) as cpool,
            tc.tile_pool(name="wpool", bufs=1) as wpool,
        ):
            WT = wpool.tile([P, NT, N], fp32, tag="WT")
            MEGA = mpool.tile([P, NT, H, N], fp32, tag="MEGA")
            negone = cpool.tile([P, 1], fp32, tag="negone")
            nc.vector.memset(negone[:, :], -1.0)

            # Two loads total: tile 0 first (compute starts after ~2MB),
            # then tiles 1-7 in one transfer overlapping tile-0 compute.
            sig_r = sigma.rearrange("(t p) i j -> p t i j", p=P)
            nc.sync.dma_start(out=MEGA[:, 0, 0:N, :], in_=sig_r[:, 0])
            nc.sync.dma_start(out=MEGA[:, 1:NT, 0:N, :], in_=sig_r[:, 1:NT])

            for t in range(NT):
                X = MEGA[:, t]
                R = spool.tile([P, N], fp32, tag="R")
                PT = ptpool.tile([P, N, N - 1], fp32, tag="PT")

                # Sole carrier of this tile's DMA wait; rewrites X so all
                # later VectorE instructions depend on it via same-engine
                # program order only.
                nc.vector.tensor_copy(X[:, 0:N, :], X[:, 0:N, :])
                # rhs row (ones)
                nc.vector.memset(X[:, N, :], 1.0)

                # ---- forward elimination (GE on rows of S; c rides along) ----
                for k in range(N):
                    nc.vector.reciprocal(R[:, k:k + 1], X[:, k, k:k + 1])
                    if k == N - 1:
                        continue
                    # nt = -(r_k * c[k])
                    nt = spool.tile([P, 1], fp32, tag="nt")
                    nc.vector.scalar_tensor_tensor(
                        out=nt[:, :],
                        in0=X[:, N, k:k + 1],
                        scalar=R[:, k:k + 1],
                        in1=negone[:, :],
                        op0=Alu.mult,
                        op1=Alu.mult,
                    )
                    nr = N - 1 - k    # rows k+1 .. 63
                    ncol = N - 1 - k  # cols k+1 .. 63
                    colv = X[:, k + 1:N, k:k + 1].broadcast_to([P, nr, ncol])
                    rowv = X[:, k:k + 1, k + 1:N].broadcast_to([P, nr, ncol])
                    # PT[i,j] = X[i,k] * (1/X[k,k]) * X[k,j]
                    nc.vector.scalar_tensor_tensor(
                        out=PT[:, 0:nr, 0:ncol],
                        in0=colv,
                        scalar=R[:, k:k + 1],
                        in1=rowv,
                        op0=Alu.mult,
                        op1=Alu.mult,
                    )
                    # X[i,j] -= PT[i,j]
                    nc.vector.tensor_sub(
                        X[:, k + 1:N, k + 1:N],
                        X[:, k + 1:N, k + 1:N],
                        PT[:, 0:nr, 0:ncol],
                    )
                    # c[i] -= m_ik * c[k]  (i = k+1 .. 63)
                    nc.vector.scalar_tensor_tensor(
                        out=X[:, N, k + 1:N],
                        in0=X[:, k + 1:N, k],
                        scalar=nt[:, :],
                        in1=X[:, N, k + 1:N],
                        op0=Alu.mult,
                        op1=Alu.add,
                    )

                # ---- back substitution (j descending; zn = -z in rhs row) ----
                for j in range(N - 1, -1, -1):
                    nc.vector.scalar_tensor_tensor(
                        out=X[:, N, j:j + 1],
                        in0=X[:, N, j:j + 1],
                        scalar=R[:, j:j + 1],
                        in1=negone[:, :],
                        op0=Alu.mult,
                        op1=Alu.mult,
                    )
                    if j == 0:
                        continue
                    # c[j'] += U[j',j] * zn[j]  (j' < j)
                    nc.vector.scalar_tensor_tensor(
                        out=X[:, N, 0:j],
                        in0=X[:, 0:j, j],
                        scalar=X[:, N, j:j + 1],
                        in1=X[:, N, 0:j],
                        op0=Alu.mult,
                        op1=Alu.add,
                    )

                # ---- normalize: w = zn / sum(zn) (sign cancels) ----
                s = spool.tile([P, 1], fp32, tag="s")
                rs = spool.tile([P, 1], fp32, tag="rs")
                nc.vector.tensor_reduce(
                    out=s[:, :],
                    in_=X[:, N, :],
                    axis=mybir.AxisListType.X,
                    op=Alu.add,
                )
                nc.vector.reciprocal(rs[:, :], s[:, :])
                nc.vector.tensor_scalar_mul(WT[:, t, :], X[:, N, :], rs[:, :])

            nc.sync.dma_start(
                out=wout.rearrange("(t p) n -> p t n", p=P), in_=WT[:, :, :]
            )

    return nc


def kernel(sigma: np.ndarray) -> np.ndarray:
    global LAST_EXEC_NS
    import time

    from concourse.bass_utils import run_bass_kernel_spmd

    if "nc" not in _CACHE:
        _CACHE["nc"] = _build_program()
    nc = _CACHE["nc"]

    sigma = np.ascontiguousarray(sigma, dtype=np.float32)
    shards = sigma.reshape(NCORES, BPC, N, N)
    in_maps = [{"sigma": shards[i]} for i in range(NCORES)]

    res = run_bass_kernel_spmd(
        nc, in_maps, core_ids=list(range(NCORES))
    )

    if os.environ.get("BASS_KERNEL_TIME", "0") == "1":
        t0 = time.perf_counter()
        res = run_bass_kernel_spmd(
            nc, in_maps, core_ids=list(range(NCORES))
        )
        LAST_EXEC_NS = int((time.perf_counter() - t0) * 1e9)

    out = np.concatenate([res.results[i]["w"] for i in range(NCORES)], axis=0)
    return out.reshape(B, N, 1).astype(np.float32)

